# revision 5
# baseline (speedup 1.0000x reference)
"""Trainium2 Bass kernel for the CustomODELoss problem.

Full inputs:
    predicted_solution_batch [4096, 8192] f32
    target_solution_batch    [4096, 8192] f32
    c_input_batch            [4096]       f32
    x_eval_points            [8192]       f32   (uniform grid on [0, 1])

loss = mean((pred - target)^2)
     + mean((pred[r, idx_r] - 1)^2)
     + mean(((pred[r, idx_p] - pred[r, idx_m]) / ((idx_p - idx_m) * dx))^2)
where idx_r = argmin_j |x_j - c_r| (first index on ties).

Sharding: data-parallel over the batch dim, 512 rows per core on 8 cores.

Device-side work is the memory-bound part only: stream the pred/targ
slices once (sum of squared differences), plus one tiny 3-wide indirect
gather per row for the f(c) / f'(c) terms.  The per-row grid index
resolve (argmin over the uniform grid) runs on HOST numpy over the tiny
c / x_eval inputs with bit-identical f32 semantics to the reference
(same |x - c| values, same first-index tie-break); the device receives
precomputed gather offsets plus select/finite-difference WEIGHTS, so
f(c) = sum(w_fpc * window) and f'(c) = sum(w_fpp * window) are two
multiply+reduce pairs.

Streaming layout (the perf-critical part; per core 2 x 16 MiB):
  - full-width [128, 8192] tiles (32 KiB contiguous per partition row)
    for the first 3 row blocks, tapered tail (4096/2048/1024/1024) so
    the last load -> subtract -> square pipeline tail is short;
  - pred rides the SP HWDGE ring (nc.sync), targ the Activation HWDGE
    ring (nc.scalar): two descriptor streams keep all 16 DMA engines
    fed (one ring leaves ~35ns/packet head-of-ring gaps = ~14% idle);
  - subtract runs in place into the targ tile and the ACT square in
    place again, so there is no third streaming pool;
  - total device program is ~50 instructions (vs ~630 for a 20-pair
    schedule with on-device index resolve) which shrinks the
    event-semaphore init/teardown scaffolding that the framework emits
    per instruction (~11us of teardown at 630 instructions).

The device emits per-partition partial sums [128, 3]; the host sums the
8 cores' partials in f64 and forms the three means.
"""

import numpy as np

import concourse.bacc as bacc
import concourse.bass as bass
import concourse.mybir as mybir
from concourse import tile
from concourse.bass_utils import run_bass_kernel_spmd

F32 = mybir.dt.float32
I32 = mybir.dt.int32
OP = mybir.AluOpType

B = 4096
N = 8192
NCORES = 8
BL = B // NCORES          # rows per core = 512
P = 128                   # SBUF partitions
RB = BL // P              # row groups per partition = 4
W = 3                     # gather window width

# Streaming schedule: (col_start, width) per row block 0..3.  Full-width
# tiles maximize DRAM contiguity (32 KiB per partition row per packet);
# the last row block tapers so the serial tail (last load -> subtract ->
# square -> reduce -> store) is ~1.5us instead of ~9us.
TILES = [
    (0, 0, N),
    (1, 0, N),
    (2, 0, N),
    (3, 0, 4096),
    (3, 4096, 2048),
    (3, 6144, 1024),
    (3, 7168, 1024),
]
NT = len(TILES)
FT = N                    # pool tile free dim (max width)


def build_nc(debug=False):
    # Bacc (not plain Bass): its compile pipeline runs
    # generate_event_semaphores, which splits multi-sem waits into separate
    # event instructions — TRN2 allows at most 1 embedded wait per
    # instruction, and walrus codegen rejects the unsplit form.
    nc = bacc.Bacc()

    pred = nc.dram_tensor("pred", [BL, N], F32, kind="ExternalInput")
    targ = nc.dram_tensor("targ", [BL, N], F32, kind="ExternalInput")
    # host-computed: flat gather offsets (row*N + clip(idx-1, 0, N-3)),
    # row r = p*RB + q
    ints = nc.dram_tensor("ints", [P, RB], I32, kind="ExternalInput")
    # host-computed weights: [:, 0:12] = f(c) one-hot select,
    # [:, 12:24] = f'(c) (+1/-1)/denom finite-difference weights,
    # both laid out [128, RB*W]
    wts = nc.dram_tensor("wts", [P, 2 * RB * W], F32, kind="ExternalInput")
    partials = nc.dram_tensor("partials", [P, 3], F32, kind="ExternalOutput")
    if debug:
        dbg = nc.dram_tensor("dbg", [P, 24], F32, kind="ExternalOutput")

    def view3(t):  # [128, 12] AP -> [128, 4, 3] AP
        return t.rearrange("p (q k) -> p q k", k=W)

    with tile.TileContext(nc) as tc:
        with (
            tc.tile_pool(name="ppool", bufs=2) as ppool,
            tc.tile_pool(name="tpool", bufs=2) as tpool,
            tc.tile_pool(name="pb", bufs=1) as pb,
        ):
            # tiny loads ride the ACT ring ahead of targ so the SP ring
            # opens directly with the first big pred load
            ints_t = pb.tile([P, RB], I32)
            nc.scalar.dma_start(ints_t[:], ints[:, :])
            wts_t = pb.tile([P, 2 * RB * W], F32)
            nc.scalar.dma_start(wts_t[:], wts[:, :])

            parts = pb.tile([P, NT], F32)
            po = pb.tile([P, 3], F32)

            pt = [None] * NT
            tt = [None] * NT

            def load(k):
                rb, cs, w = TILES[k]
                rs = rb * P
                pt[k] = ppool.tile([P, FT], F32, name="pt")
                nc.sync.dma_start(pt[k][:, :w], pred[rs:rs + P, cs:cs + w])
                tt[k] = tpool.tile([P, FT], F32, name="tt")
                nc.scalar.dma_start(tt[k][:, :w], targ[rs:rs + P, cs:cs + w])

            def compute(k):
                _, _, w = TILES[k]
                # diff in place into the targ tile, square+row-sum in place
                nc.vector.tensor_tensor(out=tt[k][:, :w], in0=pt[k][:, :w],
                                        in1=tt[k][:, :w], op=OP.subtract)
                nc.scalar.activation(
                    out=tt[k][:, :w], in_=tt[k][:, :w],
                    func=mybir.ActivationFunctionType.Square,
                    accum_out=parts[:, k:k + 1],
                )

            # fill the 2-deep pipe
            load(0)
            load(1)

            # gathers: 3-wide pred window per row via SWDGE; one offset
            # per partition per instruction (HW honors only one)
            pw = pb.tile([P, RB * W], F32)
            for q in range(RB):
                nc.gpsimd.indirect_dma_start(
                    out=pw[:, W * q:W * q + W], out_offset=None,
                    in_=pred[:, :],
                    in_offset=bass.IndirectOffsetOnAxis(
                        ap=ints_t[:, q:q + 1], axis=1),
                )

            for k in range(NT):
                compute(k)
                if k == 0:
                    # f(c) / f'(c): weighted 3-window sums; off the
                    # streaming critical path, runs while block 1 lands
                    sel = pb.tile([P, RB * W], F32)
                    nc.vector.tensor_tensor(out=sel[:], in0=wts_t[:, :RB * W],
                                            in1=pw[:], op=OP.mult)
                    fpc = pb.tile([P, RB], F32)
                    nc.vector.reduce_sum(out=fpc[:], in_=view3(sel[:]),
                                         axis=mybir.AxisListType.X)
                    fdw = pb.tile([P, RB * W], F32)
                    nc.vector.tensor_tensor(out=fdw[:], in0=wts_t[:, RB * W:],
                                            in1=pw[:], op=OP.mult)
                    fpp = pb.tile([P, RB], F32)
                    nc.vector.reduce_sum(out=fpp[:], in_=view3(fdw[:]),
                                         axis=mybir.AxisListType.X)
                    # term2: (f(c) - 1)^2; term3: f'(c)^2
                    fpm1 = pb.tile([P, RB], F32)
                    nc.vector.tensor_scalar(out=fpm1[:], in0=fpc[:],
                                            scalar1=-1.0, scalar2=None,
                                            op0=OP.add)
                    sq2 = pb.tile([P, RB], F32)
                    nc.scalar.activation(out=sq2[:], in_=fpm1[:],
                                         func=mybir.ActivationFunctionType.Square,
                                         accum_out=po[:, 1:2])
                    sq3 = pb.tile([P, RB], F32)
                    nc.scalar.activation(out=sq3[:], in_=fpp[:],
                                         func=mybir.ActivationFunctionType.Square,
                                         accum_out=po[:, 2:3])
                    if debug:
                        dbt = pb.tile([P, 24], F32)
                        nc.vector.tensor_copy(out=dbt[:, 0:12], in_=pw[:])
                        nc.vector.tensor_copy(out=dbt[:, 12:16], in_=fpc[:])
                        nc.vector.tensor_copy(out=dbt[:, 16:20], in_=fpp[:])
                        offf = pb.tile([P, RB], F32)
                        nc.vector.tensor_copy(out=offf[:], in_=ints_t[:])
                        nc.vector.tensor_copy(out=dbt[:, 20:24], in_=offf[:])
                        nc.sync.dma_start(dbg[:, :], dbt[:])
                if k + 2 < NT:
                    load(k + 2)

            nc.vector.reduce_sum(out=po[:, 0:1], in_=parts[:],
                                 axis=mybir.AxisListType.X)
            nc.sync.dma_start(partials[:, :], po[:])

    return nc


_NC_CACHE = None


def _get_nc():
    global _NC_CACHE
    if _NC_CACHE is None:
        nc = build_nc()
        # Bacc runs its compile pipeline (register alloc, sync-wait
        # splitting) in finalize; the PJRT exec path requires it.
        nc.finalize()
        _NC_CACHE = nc
    return _NC_CACHE


def _host_index_prep(c, x):
    """Exact replication of the reference index math on the tiny inputs.

    idx = argmin_j |x_j - c_r| with numpy f32 ops — bit-identical values
    and the same first-index tie-break as jnp.argmin on CPU.
    Returns flat gather offsets into each core's [BL, N] pred slice and
    the f(c)/f'(c) window weights.
    """
    Bfull = c.shape[0]
    idx = np.empty(Bfull, dtype=np.int64)
    CH = 512
    for s in range(0, Bfull, CH):
        e = min(s + CH, Bfull)
        d = np.abs(x[None, :] - c[s:e, None])  # f32
        idx[s:e] = np.argmin(d, axis=1)
    dx = np.float32(x[1]) - np.float32(x[0])

    ip = np.minimum(idx + 1, N - 1)
    im = np.maximum(idx - 1, 0)
    s3 = np.clip(idx - 1, 0, N - W)           # window start
    p0 = (idx - s3).astype(np.int64)          # positions in window
    pm = (im - s3).astype(np.int64)
    pp = (ip - s3).astype(np.int64)
    denom = (ip - im).astype(np.float32) * dx
    rden = np.float32(1.0) / denom

    rows = np.arange(Bfull)
    wfpc = np.zeros((Bfull, W), dtype=np.float32)
    wfpc[rows, p0] = 1.0
    wfpp = np.zeros((Bfull, W), dtype=np.float32)
    # += not =: at the boundary pm == pp never happens (pm<p0<=pp or
    # pm<=p0<pp), but keep the general form cheap and safe
    np.add.at(wfpp, (rows, pp), rden)
    np.add.at(wfpp, (rows, pm), -rden)

    row_in_core = np.arange(Bfull) % BL
    offs = (row_in_core * N + s3).astype(np.int32)
    return offs, wfpc, wfpp


def make_in_maps(predicted_solution_batch, target_solution_batch,
                 c_input_batch, x_eval_points):
    pred = np.ascontiguousarray(predicted_solution_batch, dtype=np.float32)
    targ = np.ascontiguousarray(target_solution_batch, dtype=np.float32)
    c = np.ascontiguousarray(c_input_batch, dtype=np.float32)
    x = np.ascontiguousarray(x_eval_points, dtype=np.float32)
    offs, wfpc, wfpp = _host_index_prep(c, x)

    in_maps = []
    for i in range(NCORES):
        sl = slice(i * BL, (i + 1) * BL)
        # row r in core = p*RB + q  ->  [P, RB] / [P, RB*W] layouts
        wf1 = wfpc[sl].reshape(P, RB * W)
        wf2 = wfpp[sl].reshape(P, RB * W)
        in_maps.append({
            "pred": pred[sl],
            "targ": targ[sl],
            "ints": offs[sl].reshape(P, RB),
            "wts": np.ascontiguousarray(np.concatenate([wf1, wf2], axis=1)),
        })
    return in_maps


def reduce_partials(results):
    s = np.zeros(3, dtype=np.float64)
    for r in results:
        s += r["partials"].astype(np.float64).sum(axis=0)
    loss = s[0] / (B * N) + s[1] / B + s[2] / B
    return np.float32(loss)


def kernel(predicted_solution_batch, target_solution_batch,
           c_input_batch, x_eval_points):
    nc = _get_nc()
    in_maps = make_in_maps(predicted_solution_batch, target_solution_batch,
                           c_input_batch, x_eval_points)
    res = run_bass_kernel_spmd(nc, in_maps, core_ids=list(range(NCORES)))
    return reduce_partials(res.results)


# revision 6
# speedup vs baseline: 1.0566x; 1.0566x over previous
"""Trainium2 Bass kernel for the CustomODELoss problem.

Full inputs:
    predicted_solution_batch [4096, 8192] f32
    target_solution_batch    [4096, 8192] f32
    c_input_batch            [4096]       f32
    x_eval_points            [8192]       f32   (uniform grid on [0, 1])

loss = mean((pred - target)^2)
     + mean((pred[r, idx_r] - 1)^2)
     + mean(((pred[r, idx_p] - pred[r, idx_m]) / ((idx_p - idx_m) * dx))^2)
where idx_r = argmin_j |x_j - c_r| (first index on ties).

Sharding: data-parallel over the batch dim, 512 rows per core on 8 cores.

Device-side work is the memory-bound part only: stream the pred/targ
slices once (sum of squared differences), plus one tiny 3-wide indirect
gather per row for the f(c) / f'(c) terms.  The per-row grid index
resolve (argmin over the uniform grid) runs on HOST numpy over the tiny
c / x_eval inputs with bit-identical f32 semantics to the reference
(same |x - c| values, same first-index tie-break); the device receives
precomputed gather offsets plus select/finite-difference WEIGHTS, so
f(c) = sum(w_fpc * window) and f'(c) = sum(w_fpp * window) are two
multiply+reduce pairs.

Streaming design notes (measured on HW traces):
  - pred rides the SP HWDGE ring (nc.sync), targ the Activation HWDGE
    ring (nc.scalar).  The 16 DMA engines strictly alternate between
    the two rings' packet queues, which hides the ~35ns/packet
    head-of-ring gap that left the engines ~14% idle on a single ring.
  - 2048-wide tiles (8 KiB per partition row per packet): DVE
    tensor_tensor at this width runs at ~214 G elem/s vs ~120 G at
    8192-wide/in-place, and small packets keep any ring-shared tiny
    transfer's round-robin latency low.
  - the tiny ints/wts loads and the pw gathers go through the GPSIMD
    SOFTWARE DGE ring: a [128, small] load is 128 separate packets
    that round-robin 1:1 with streaming packets per engine, so putting
    them on a HW ring ahead of the stream stalls that ring for
    ~(128/16)*packet_dur; on the SWDGE ring they only add ~3us of
    latency to the (off-critical-path) gather chain.
  - all compute runs on DVE (subtract, then square via
    scalar_tensor_tensor (d*1)*d with accum_out row-sums), so the
    Scalar engine's instruction stream is pure DMA triggers and the
    ~1.3us ACT_TABLE_LOAD never gates the targ ring.
  - last row block tapers (3x2048 + 2x1024) so the serial tail
    (last load -> subtract -> square -> reduce -> store) is ~2us.

The device emits per-partition partial sums [128, 3]; the host sums the
8 cores' partials in f64 and forms the three means.
"""

import numpy as np

import concourse.bacc as bacc
import concourse.bass as bass
import concourse.mybir as mybir
from concourse import tile
from concourse.bass_utils import run_bass_kernel_spmd

F32 = mybir.dt.float32
I32 = mybir.dt.int32
OP = mybir.AluOpType

B = 4096
N = 8192
NCORES = 8
BL = B // NCORES          # rows per core = 512
P = 128                   # SBUF partitions
RB = BL // P              # row groups per partition = 4
W = 3                     # gather window width
FT = 2048                 # streaming tile width

# (row_block, col_start, width); last row block tapers
TILES = []
for _rb in range(RB):
    if _rb < RB - 1:
        TILES += [(_rb, c * FT, FT) for c in range(N // FT)]
    else:
        TILES += [(_rb, 0, 2048), (_rb, 2048, 2048), (_rb, 4096, 2048),
                  (_rb, 6144, 1024), (_rb, 7168, 1024)]
NT = len(TILES)           # 17
NFILL = 6                 # pipeline fill depth (pairs in flight)


def build_nc(debug=False):
    # Bacc (not plain Bass): its compile pipeline runs
    # generate_event_semaphores, which splits multi-sem waits into separate
    # event instructions — TRN2 allows at most 1 embedded wait per
    # instruction, and walrus codegen rejects the unsplit form.
    nc = bacc.Bacc()

    pred = nc.dram_tensor("pred", [BL, N], F32, kind="ExternalInput")
    targ = nc.dram_tensor("targ", [BL, N], F32, kind="ExternalInput")
    # host-computed: flat gather offsets (row*N + clip(idx-1, 0, N-3)),
    # row r = p*RB + q
    ints = nc.dram_tensor("ints", [P, RB], I32, kind="ExternalInput")
    # host-computed weights: [:, 0:12] = f(c) one-hot select,
    # [:, 12:24] = f'(c) (+1/-1)/denom finite-difference weights,
    # both laid out [128, RB*W]
    wts = nc.dram_tensor("wts", [P, 2 * RB * W], F32, kind="ExternalInput")
    partials = nc.dram_tensor("partials", [P, 3], F32, kind="ExternalOutput")
    if debug:
        dbg = nc.dram_tensor("dbg", [P, 24], F32, kind="ExternalOutput")

    def view3(t):  # [128, 12] AP -> [128, 4, 3] AP
        return t.rearrange("p (q k) -> p q k", k=W)

    with tile.TileContext(nc) as tc:
        with (
            tc.tile_pool(name="ppool", bufs=NFILL) as ppool,
            tc.tile_pool(name="tpool", bufs=NFILL) as tpool,
            tc.tile_pool(name="dpool", bufs=4) as dpool,
            tc.tile_pool(name="pb", bufs=1) as pb,
        ):
            # tiny loads via SWDGE so neither HW ring stalls on them
            ints_t = pb.tile([P, RB], I32)
            nc.gpsimd.dma_start(ints_t[:], ints[:, :])
            wts_t = pb.tile([P, 2 * RB * W], F32)
            nc.gpsimd.dma_start(wts_t[:], wts[:, :])

            parts = pb.tile([P, NT], F32)
            po = pb.tile([P, 3], F32)

            pt = [None] * NT
            tt = [None] * NT

            def load(k):
                rb, cs, w = TILES[k]
                rs = rb * P
                pt[k] = ppool.tile([P, FT], F32, name="pt")
                nc.sync.dma_start(pt[k][:, :w], pred[rs:rs + P, cs:cs + w])
                tt[k] = tpool.tile([P, FT], F32, name="tt")
                nc.scalar.dma_start(tt[k][:, :w], targ[rs:rs + P, cs:cs + w])

            def compute(k):
                _, _, w = TILES[k]
                dt = dpool.tile([P, FT], F32, name="dt")
                nc.vector.tensor_tensor(out=dt[:, :w], in0=pt[k][:, :w],
                                        in1=tt[k][:, :w], op=OP.subtract)
                # square + per-partition row-sum on DVE; out overwrites the
                # (dead) pred tile so no extra pool is needed
                nc.vector.scalar_tensor_tensor(
                    out=pt[k][:, :w], in0=dt[:, :w], scalar=1.0,
                    in1=dt[:, :w], op0=OP.mult, op1=OP.mult,
                    accum_out=parts[:, k:k + 1],
                )

            for k in range(NFILL):
                load(k)

            # gathers: 3-wide pred window per row via SWDGE; one offset
            # per partition per instruction (HW honors only one)
            pw = pb.tile([P, RB * W], F32)
            for q in range(RB):
                nc.gpsimd.indirect_dma_start(
                    out=pw[:, W * q:W * q + W], out_offset=None,
                    in_=pred[:, :],
                    in_offset=bass.IndirectOffsetOnAxis(
                        ap=ints_t[:, q:q + 1], axis=1),
                )

            for k in range(NT):
                compute(k)
                if k == 0:
                    # f(c) / f'(c): weighted 3-window sums; off the
                    # streaming critical path
                    sel = pb.tile([P, RB * W], F32)
                    nc.vector.tensor_tensor(out=sel[:], in0=wts_t[:, :RB * W],
                                            in1=pw[:], op=OP.mult)
                    fpc = pb.tile([P, RB], F32)
                    nc.vector.reduce_sum(out=fpc[:], in_=view3(sel[:]),
                                         axis=mybir.AxisListType.X)
                    fdw = pb.tile([P, RB * W], F32)
                    nc.vector.tensor_tensor(out=fdw[:], in0=wts_t[:, RB * W:],
                                            in1=pw[:], op=OP.mult)
                    fpp = pb.tile([P, RB], F32)
                    nc.vector.reduce_sum(out=fpp[:], in_=view3(fdw[:]),
                                         axis=mybir.AxisListType.X)
                    # term2: (f(c) - 1)^2; term3: f'(c)^2
                    fpm1 = pb.tile([P, RB], F32)
                    nc.vector.tensor_scalar(out=fpm1[:], in0=fpc[:],
                                            scalar1=-1.0, scalar2=None,
                                            op0=OP.add)
                    sq2 = pb.tile([P, RB], F32)
                    nc.vector.scalar_tensor_tensor(
                        out=sq2[:], in0=fpm1[:], scalar=1.0, in1=fpm1[:],
                        op0=OP.mult, op1=OP.mult, accum_out=po[:, 1:2])
                    sq3 = pb.tile([P, RB], F32)
                    nc.vector.scalar_tensor_tensor(
                        out=sq3[:], in0=fpp[:], scalar=1.0, in1=fpp[:],
                        op0=OP.mult, op1=OP.mult, accum_out=po[:, 2:3])
                    if debug:
                        dbt = pb.tile([P, 24], F32)
                        nc.vector.tensor_copy(out=dbt[:, 0:12], in_=pw[:])
                        nc.vector.tensor_copy(out=dbt[:, 12:16], in_=fpc[:])
                        nc.vector.tensor_copy(out=dbt[:, 16:20], in_=fpp[:])
                        offf = pb.tile([P, RB], F32)
                        nc.vector.tensor_copy(out=offf[:], in_=ints_t[:])
                        nc.vector.tensor_copy(out=dbt[:, 20:24], in_=offf[:])
                        nc.sync.dma_start(dbg[:, :], dbt[:])
                if k + NFILL < NT:
                    load(k + NFILL)

            nc.vector.reduce_sum(out=po[:, 0:1], in_=parts[:],
                                 axis=mybir.AxisListType.X)
            nc.sync.dma_start(partials[:, :], po[:])

    return nc


_NC_CACHE = None


def _get_nc():
    global _NC_CACHE
    if _NC_CACHE is None:
        nc = build_nc()
        # Bacc runs its compile pipeline (register alloc, sync-wait
        # splitting) in finalize; the PJRT exec path requires it.
        nc.finalize()
        _NC_CACHE = nc
    return _NC_CACHE


def _host_index_prep(c, x):
    """Exact replication of the reference index math on the tiny inputs.

    idx = argmin_j |x_j - c_r| with numpy f32 ops — bit-identical values
    and the same first-index tie-break as jnp.argmin on CPU.
    Returns flat gather offsets into each core's [BL, N] pred slice and
    the f(c)/f'(c) window weights.
    """
    Bfull = c.shape[0]
    idx = np.empty(Bfull, dtype=np.int64)
    CH = 512
    for s in range(0, Bfull, CH):
        e = min(s + CH, Bfull)
        d = np.abs(x[None, :] - c[s:e, None])  # f32
        idx[s:e] = np.argmin(d, axis=1)
    dx = np.float32(x[1]) - np.float32(x[0])

    ip = np.minimum(idx + 1, N - 1)
    im = np.maximum(idx - 1, 0)
    s3 = np.clip(idx - 1, 0, N - W)           # window start
    p0 = (idx - s3).astype(np.int64)          # positions in window
    pm = (im - s3).astype(np.int64)
    pp = (ip - s3).astype(np.int64)
    denom = (ip - im).astype(np.float32) * dx
    rden = np.float32(1.0) / denom

    rows = np.arange(Bfull)
    wfpc = np.zeros((Bfull, W), dtype=np.float32)
    wfpc[rows, p0] = 1.0
    wfpp = np.zeros((Bfull, W), dtype=np.float32)
    # += not =: pm and pp never collide (pm < pp always since ip > im),
    # but keep the accumulate form cheap and safe
    np.add.at(wfpp, (rows, pp), rden)
    np.add.at(wfpp, (rows, pm), -rden)

    row_in_core = np.arange(Bfull) % BL
    offs = (row_in_core * N + s3).astype(np.int32)
    return offs, wfpc, wfpp


def make_in_maps(predicted_solution_batch, target_solution_batch,
                 c_input_batch, x_eval_points):
    pred = np.ascontiguousarray(predicted_solution_batch, dtype=np.float32)
    targ = np.ascontiguousarray(target_solution_batch, dtype=np.float32)
    c = np.ascontiguousarray(c_input_batch, dtype=np.float32)
    x = np.ascontiguousarray(x_eval_points, dtype=np.float32)
    offs, wfpc, wfpp = _host_index_prep(c, x)

    in_maps = []
    for i in range(NCORES):
        sl = slice(i * BL, (i + 1) * BL)
        # row r in core = p*RB + q  ->  [P, RB] / [P, RB*W] layouts
        wf1 = wfpc[sl].reshape(P, RB * W)
        wf2 = wfpp[sl].reshape(P, RB * W)
        in_maps.append({
            "pred": pred[sl],
            "targ": targ[sl],
            "ints": offs[sl].reshape(P, RB),
            "wts": np.ascontiguousarray(np.concatenate([wf1, wf2], axis=1)),
        })
    return in_maps


def reduce_partials(results):
    s = np.zeros(3, dtype=np.float64)
    for r in results:
        s += r["partials"].astype(np.float64).sum(axis=0)
    loss = s[0] / (B * N) + s[1] / B + s[2] / B
    return np.float32(loss)


def kernel(predicted_solution_batch, target_solution_batch,
           c_input_batch, x_eval_points):
    nc = _get_nc()
    in_maps = make_in_maps(predicted_solution_batch, target_solution_batch,
                           c_input_batch, x_eval_points)
    res = run_bass_kernel_spmd(nc, in_maps, core_ids=list(range(NCORES)))
    return reduce_partials(res.results)
